# revision 27
# baseline (speedup 1.0000x reference)
"""Trainium2 Bass kernel for nn_AttentionHead (B=2, S=2048, D=768, H=12).

Sharding: 8 cores = 2 batches x 4 head-groups (3 heads each).
Per core: QKV projection for its heads (transposed layout), causal
attention with softmax over the QUERY axis (reference peculiarity:
softmax dim=-2, scaled by sqrt(d_model)), AllGather of per-head outputs
within each batch's 4-core group, then a column-slice of the output
projection.  Host only slices / transposes / concatenates.

Layout / scheduling choices:
  - Scores are built transposed: S_T[k, q] so the softmax axis (q) is
    the SBUF free axis; ScalarE exp computes the row sums for free via
    accum_out.  The per-k normalizer is folded into V ("V'") so the
    attn @ v matmul consumes raw exp scores.
  - Causal structure (checked on host) skips ~40% of score blocks; the
    diagonal triangle is masked by a DVE add of -1e30 into PSUM.
  - x is DMA'd in S-halves and the QKV chunks are emitted dt-major so
    the projection matmuls pipeline with the DMA arrivals.
  - Waves are software-pipelined: AV(step-1) is emitted after
    scores(step), so the PE stays busy while ScalarE runs the exps
    (keeps the HAM clock at full rate).
  - waveB's AV accumulator is folded to [128, 1024] (q-halves in
    partition halves) freeing 2 PSUM banks -> 3 strip buffers.
  - Output projection is chunked per 512 columns and interleaved into
    waveA right after each chunk's AllGather lands.
"""

import contextlib
import math

import numpy as np

B, S, D, H, DH = 2, 2048, 768, 12, 64
NCORES = 8
GROUPS = 4  # head-groups per batch
HPG = 3  # heads per group
EPG = HPG * DH  # 192
SCALE = 1.0 / math.sqrt(D)
NEG = -1.0e30

_cache = {}


NMAX = 1024  # bf16 moving-operand max per matmul
SH = 1024  # S-half for x DMA staging


def _build(causal: bool):
    import concourse.bacc as bacc
    import concourse.mybir as mybir
    from concourse import tile

    f32 = mybir.dt.float32
    bf16 = mybir.dt.bfloat16
    EXP = mybir.ActivationFunctionType.Exp

    nc = bacc.Bacc("TRN2", target_bir_lowering=False, debug=False, num_devices=NCORES)

    xT = nc.dram_tensor("xT", [D, S], f32, kind="ExternalInput")
    wqk = nc.dram_tensor("wqk", [D, 384], f32, kind="ExternalInput")
    wv = nc.dram_tensor("wv", [D, EPG], f32, kind="ExternalInput")
    bqkc = nc.dram_tensor("bqkc", [384, 1], f32, kind="ExternalInput")
    bv4 = nc.dram_tensor("bv4", [1, 4 * EPG], f32, kind="ExternalInput")
    wout = nc.dram_tensor("wout", [D, EPG], f32, kind="ExternalInput")
    boutc = nc.dram_tensor("boutc", [EPG, 1], f32, kind="ExternalInput")
    tri = nc.dram_tensor("tri", [128, 128], f32, kind="ExternalInput")
    triT = nc.dram_tensor("triT", [128, 128], f32, kind="ExternalInput")
    ident = nc.dram_tensor("ident", [128, 128], f32, kind="ExternalInput")
    out = nc.dram_tensor("out", [EPG, S], f32, kind="ExternalOutput")

    ag_inA = [nc.dram_tensor(f"ag_inA{f}", [128, 512], bf16) for f in range(4)]
    ag_outA = [nc.dram_tensor(f"ag_outA{f}", [512, 512], bf16) for f in range(4)]
    # final flush split in two 256-col AllGathers to shorten the tail
    ag_inA3h = [nc.dram_tensor(f"ag_inA3h{i}", [128, 256], bf16) for i in range(2)]
    ag_outA3h = [nc.dram_tensor(f"ag_outA3h{i}", [512, 256], bf16) for i in range(2)]
    ag_inB = [nc.dram_tensor(f"ag_inB{f}", [64, 1024], bf16) for f in range(2)]
    ag_outB = [nc.dram_tensor(f"ag_outB{f}", [256, 1024], bf16) for f in range(2)]
    bar_in = nc.dram_tensor("bar_in", [1, 8], bf16)
    bar_out = nc.dram_tensor("bar_out", [4, 8], bf16)

    groups = [[0, 1, 2, 3], [4, 5, 6, 7]]

    with tile.TileContext(nc) as tc:
        with contextlib.ExitStack() as ctx:
            const_p = ctx.enter_context(tc.tile_pool(name="const", bufs=1))
            w_p = ctx.enter_context(tc.tile_pool(name="w", bufs=6))
            qk_p = ctx.enter_context(tc.tile_pool(name="qk", bufs=1))
            v_p = ctx.enter_context(tc.tile_pool(name="v", bufs=1))
            e_p = ctx.enter_context(tc.tile_pool(name="e", bufs=13))
            st_p = ctx.enter_context(tc.tile_pool(name="stat", bufs=24))
            vp_p = ctx.enter_context(tc.tile_pool(name="vp", bufs=10))
            atn_p = ctx.enter_context(tc.tile_pool(name="atn", bufs=1))
            ag_p = ctx.enter_context(tc.tile_pool(name="ag", bufs=1))
            o_p = ctx.enter_context(tc.tile_pool(name="o", bufs=2))

            psS_ctx = tc.tile_pool(name="psS3", bufs=3, space="PSUM")
            psS = psS_ctx.__enter__()

            # ---- constants (tiny DMAs first) ----
            ones_f = const_p.tile([1, 512], f32)
            nc.vector.memset(ones_f[:], 1.0)
            ones = const_p.tile([1, 512], bf16)
            nc.vector.tensor_copy(ones[:], ones_f[:])
            tri_f = const_p.tile([128, 128], f32)
            nc.sync.dma_start(tri_f[:], tri[:, :])
            triT_f = const_p.tile([128, 128], f32)
            nc.sync.dma_start(triT_f[:], triT[:, :])
            tri_tb = const_p.tile([128, 128], bf16)
            nc.vector.tensor_copy(tri_tb[:], triT_f[:])
            id_f = const_p.tile([128, 128], f32)
            nc.sync.dma_start(id_f[:], ident[:, :])
            id_b = const_p.tile([128, 128], bf16)
            nc.vector.tensor_copy(id_b[:], id_f[:])

            bqk_c = const_p.tile([128, 3], f32)
            nc.sync.dma_start(bqk_c[:], bqkc[:, :].rearrange("(c p) o -> p (c o)", p=128))
            bout_c = const_p.tile([128, 2], f32)
            nc.sync.dma_start(bout_c[0:64, 1:2], boutc[128:EPG, :])
            nc.sync.dma_start(bout_c[:, 0:1], boutc[0:128, :])
            bv_f = const_p.tile([1, 4 * EPG], f32)
            nc.sync.dma_start(bv_f[:], bv4[:, :])
            bv_t = const_p.tile([1, 4 * EPG], bf16)
            nc.vector.tensor_copy(bv_t[:], bv_f[:])

            # ---- barrier AllGather: absorb cross-core launch skew early,
            # while the PE is DMA-bound anyway, so the real AllGathers
            # later don't eat a ~25us first-collective peer wait ----
            bar_t = const_p.tile([1, 8], bf16)
            nc.vector.memset(bar_t[:], 0.0)
            nc.sync.dma_start(bar_in[:, :], bar_t[:])
            nc.gpsimd.collective_compute(
                "AllGather",
                mybir.AluOpType.bypass,
                replica_groups=groups,
                ins=[bar_in.ap().opt()],
                outs=[bar_out.ap().opt()],
            )

            # ---- PE warm-up: dummy matmuls while first DMAs land ----
            warm_in = const_p.tile([128, 512], bf16)
            nc.vector.memset(warm_in[:], 0.0)
            for wi in range(8):
                wps = psS.tile([128, NMAX], f32, tag="strip")
                nc.tensor.matmul(
                    wps[:, 0:512], id_b[:], warm_in[:],
                    start=True, stop=True, skip_group_check=True,
                )

            # ---- DMA priority: per-dt (wqk, x-sh0) interleaved so the
            # first qkv group paces with arrivals; then wv, x-sh1, wout ----
            xt_ctx = tc.tile_pool(name="xt", bufs=1)
            xt_p = xt_ctx.__enter__()
            xs_ctx = tc.tile_pool(name="xs", bufs=6)
            xs_p = xs_ctx.__enter__()
            xt_t = [xt_p.tile([128, S], bf16, tag=f"xt{i}", name=f"xt{i}") for i in range(6)]
            xs_t = [xs_p.tile([128, S], f32, tag="xstg", name=f"xs{i}") for i in range(6)]
            wqk_t, wv_t, wout_t = [], [], []
            c0 = slice(0, SH)
            c1 = slice(SH, S)
            for dt_i in range(6):
                wf = w_p.tile([128, 384], f32, tag="wstg")
                nc.sync.dma_start(wf[:], wqk[dt_i * 128 : (dt_i + 1) * 128, :])
                wt = w_p.tile([128, 384], bf16, tag="wqk")
                nc.vector.tensor_copy(wt[:], wf[:])
                wqk_t.append(wt)
                nc.sync.dma_start(
                    xs_t[dt_i][:, c0], xT[dt_i * 128 : (dt_i + 1) * 128, c0]
                )
                nc.vector.tensor_copy(xt_t[dt_i][:, c0], xs_t[dt_i][:, c0])
            for dt_i in range(6):
                vf = w_p.tile([128, EPG], f32, tag="wvstg")
                nc.sync.dma_start(vf[:], wv[dt_i * 128 : (dt_i + 1) * 128, :])
                vt = w_p.tile([128, EPG], bf16, tag="wv")
                nc.vector.tensor_copy(vt[:], vf[:])
                wv_t.append(vt)
            for dt_i in range(6):
                nc.sync.dma_start(
                    xs_t[dt_i][:, c1], xT[dt_i * 128 : (dt_i + 1) * 128, c1]
                )
                nc.vector.tensor_copy(xt_t[dt_i][:, c1], xs_t[dt_i][:, c1])

            # qkv chunk for one s-half of 1024; bias folded into the copy.
            # dt-major emission across a GROUP of chunks so the matmuls
            # pipeline with the per-dt x DMA arrivals.
            def qkv_group(specs, sc):
                """specs: list of (dst_tile, c). Emits all chunks of one
                s-half dt-major; each chunk uses its own PSUM slot."""
                pts = [psS.tile([128, NMAX], f32, tag="strip", name=f"qg{ci}") for ci in range(len(specs))]
                for dt_i in range(6):
                    for pt, (dst, c) in zip(pts, specs):
                        for off in range(0, NMAX, 512):
                            nc.tensor.matmul(
                                pt[:, off : off + 512],
                                wqk_t[dt_i][:, c * 128 : (c + 1) * 128],
                                xt_t[dt_i][:, sc * NMAX + off : sc * NMAX + off + 512],
                                start=(dt_i == 0), stop=(dt_i == 5),
                                skip_group_check=True,
                            )
                for pt, (dst, c) in zip(pts, specs):
                    nc.vector.tensor_scalar_add(
                        dst[:, sc * NMAX : (sc + 1) * NMAX], pt[:], bqk_c[:, c : c + 1]
                    )

            # packed v tiles: 4 s-blocks of 128 in one PSUM slot
            def v_slot(j, vnat):
                sts = range(4 * j, 4 * j + 4)
                ptf = psS.tile([128, NMAX], f32, tag="strip")
                p = ptf[:, 0 : 4 * EPG]
                for b0 in range(0, 4 * EPG, 512):
                    bn = min(512, 4 * EPG - b0)
                    nc.tensor.matmul(
                        ptf[:, b0 : b0 + bn], ones[:, 0:128],
                        bv_t[:, b0 : b0 + bn], start=True, stop=False,
                        skip_group_check=True,
                    )
                for dt_i in range(6):
                    for jj, st_i in enumerate(sts):
                        nc.tensor.matmul(
                            ptf[:, jj * EPG : (jj + 1) * EPG],
                            xt_t[dt_i][:, st_i * 128 : (st_i + 1) * 128],
                            wv_t[dt_i][:],
                            start=False, stop=(dt_i == 5),
                            skip_group_check=True,
                        )
                nc.vector.tensor_copy(
                    vnat[:, 4 * j * EPG : (4 * j + 4) * EPG], p
                )

            # ---- upfront: all qk chunks, dt-major per s-half ----
            k01 = qk_p.tile([128, S], bf16, tag="k01")
            q01 = qk_p.tile([128, S], bf16, tag="q01")
            qk2 = qk_p.tile([128, S], bf16, tag="qk2")
            k2 = qk_p.tile([64, S], bf16, tag="k2")
            q2d = qk_p.tile([128, S], bf16, tag="q2d")
            vnat = v_p.tile([128, 16 * EPG], bf16)
            qkv_group([(qk2, 2), (q01, 1), (k01, 0)], 0)
            qkv_group([(qk2, 2), (q01, 1), (k01, 0)], 1)
            nc.gpsimd.dma_start(k2[:], qk2[64:128, :])
            nc.gpsimd.dma_start(q2d[64:128, :], qk2[0:64, :])

            # wout lands last in the DMA queue; out-proj needs it much later
            for dt_i in range(6):
                wos = w_p.tile([128, EPG], f32, tag="wostg")
                nc.sync.dma_start(wos[:], wout[dt_i * 128 : (dt_i + 1) * 128, :])
                wo = w_p.tile([128, EPG], bf16, tag="wout")
                nc.gpsimd.tensor_copy(wo[:], wos[:])
                wout_t.append(wo)

            def halves_of(ki):
                q0 = 128 * ki if causal else 0
                L = S - q0
                hs = [(q0, min(L, NMAX))]
                if L > NMAX:
                    hs.append((q0 + NMAX, L - NMAX))
                return hs

            def strip_pair(ki, srcs, hv, h0, hl):
                """Emit the two row-group S matmuls adjacently, masks, exps.
                srcs: [(kT, kbase, qT, qbase), ...] for row groups lo/hi."""
                out_tiles = []
                s_list = []
                for (kT, kb, qT, qb) in srcs:
                    s_ps = psS.tile([128, NMAX], f32, tag="strip")
                    s_list.append(s_ps)
                off = 0
                while off < hl:
                    n = min(512, hl - off)
                    for s_ps, (kT, kb, qT, qb) in zip(s_list, srcs):
                        nc.tensor.matmul(
                            s_ps[:, off : off + n],
                            kT[kb : kb + 64, ki * 128 : (ki + 1) * 128],
                            qT[qb : qb + 64, h0 + off : h0 + off + n],
                            start=True,
                            stop=True,
                            skip_group_check=True,
                        )
                    off += n
                if causal and hv == 0:
                    for s_ps in s_list:
                        nc.tensor.matmul(
                            s_ps[:, 0:128], tri_tb[:], id_b[:],
                            start=False, stop=True, skip_group_check=True,
                        )
                for s_ps in s_list:
                    et = e_p.tile([128, NMAX], bf16, tag="e")
                    acc = st_p.tile([128, 1], f32, tag="acc")
                    nc.scalar.activation(
                        et[:, 0:hl], s_ps[:, 0:hl], EXP,
                        scale=SCALE, accum_out=acc[:],
                    )
                    out_tiles.append((et, acc))
                return out_tiles

            def make_vpt(accs, ki, head):
                rcp = st_p.tile([128, 1], f32, tag="rcp")
                if len(accs) == 2:
                    ssum = st_p.tile([128, 1], f32, tag="ssum")
                    nc.vector.tensor_add(ssum[:], accs[0][:], accs[1][:])
                    nc.vector.reciprocal(rcp[:], ssum[:])
                else:
                    nc.vector.reciprocal(rcp[:], accs[0][:])
                vpt = vp_p.tile([128, 64], bf16, tag="vp")
                nc.vector.tensor_scalar_mul(
                    vpt[:],
                    vnat[:, ki * EPG + head * 64 : ki * EPG + (head + 1) * 64],
                    rcp[:],
                )
                return vpt

            agA_t = [[None] * 4 for _ in range(4)]  # [f][dt] -> [128, 512]
            for f in range(4):
                for dt_i in range(4):
                    agA_t[f][dt_i] = ag_p.tile(
                        [128, 512], bf16, tag=f"agA{f}_{dt_i}",
                        name=f"agA{f}_{dt_i}",
                    )
            agB_t = [[None] * 2 for _ in range(2)]  # [g][dt] -> [128, 1024]
            for g in range(2):
                for dt_i in range(2):
                    agB_t[g][dt_i] = ag_p.tile(
                        [128, 1024], bf16, tag=f"agB{g}_{dt_i}",
                        name=f"agB{g}_{dt_i}",
                    )

            # ---- output projection chunk: 512 cols, borrows a PSUM slot ----
            def outproj_chunk(f):
                g, part = f // 2, f % 2
                pt = psS.tile([128, NMAX], f32, tag="strip")
                for m0, mw, pc, bc in ((0, 128, 0, 0), (128, 64, 512, 1)):
                    for dt_i in range(4):
                        nc.tensor.matmul(
                            pt[0:mw, pc : pc + 512],
                            wout_t[dt_i][:, m0 : m0 + mw],
                            agA_t[f][dt_i][:, :],
                            start=(dt_i == 0), stop=False,
                            skip_group_check=True,
                        )
                    for di, dt_i in enumerate((4, 5)):
                        nc.tensor.matmul(
                            pt[0:mw, pc : pc + 512],
                            wout_t[dt_i][:, m0 : m0 + mw],
                            agB_t[g][dt_i - 4][:, 512 * part : 512 * part + 512],
                            start=False, stop=(di == 1),
                            skip_group_check=True,
                        )
                    ot = o_p.tile([128, 512], f32, tag=f"oc{bc}")
                    nc.vector.tensor_scalar_add(
                        ot[0:mw, :], pt[0:mw, pc : pc + 512], bout_c[0:mw, bc : bc + 1]
                    )
                    nc.sync.dma_start(
                        out[m0 : m0 + mw, 512 * f : 512 * (f + 1)], ot[0:mw, :]
                    )

            # ---- wave B: head 2, ki pairs in row groups; AV pipelined ----
            # av2 folded: q cols [0,1024) in partitions 0:64,
            #             q cols [1024,2048) in partitions 64:128.
            def waveB(av_ps, atn):
                def flushB(f):
                    phalf = 64 * (f // 2)
                    c0 = 512 * (f % 2)
                    nc.vector.tensor_copy(
                        atn[:, 512 * f : 512 * (f + 1)],
                        av_ps[phalf : phalf + 64, c0 : c0 + 512],
                    )
                    if f % 2 == 0:
                        return
                    g = f // 2
                    gcols = slice(1024 * g, 1024 * (g + 1))
                    nc.sync.dma_start(ag_inB[g][:, :], atn[:, gcols])
                    nc.gpsimd.collective_compute(
                        "AllGather",
                        mybir.AluOpType.bypass,
                        replica_groups=groups,
                        ins=[ag_inB[g].ap().opt()],
                        outs=[ag_outB[g].ap().opt()],
                    )
                    for dt_i in range(2):
                        nc.sync.dma_start(
                            agB_t[g][dt_i][:, :],
                            ag_outB[g][dt_i * 128 : (dt_i + 1) * 128, :],
                        )

                def scores_batch(t, filler=None):
                    kis = (2 * t, 2 * t + 1)
                    srcs = {
                        kis[0]: (k2, 0, qk2, 0),
                        kis[1]: (qk2, 64, q2d, 64),
                    }
                    ets = {ki: [] for ki in kis}
                    accs = {ki: [] for ki in kis}
                    maxhv = max(len(halves_of(ki)) for ki in kis)
                    for hv in range(maxhv):
                        # PE filler between strip batches so the exps of the
                        # previous batch overlap matmul work
                        if hv == 1 and filler is not None:
                            filler()
                            filler = None
                        batch = []
                        for ki in kis:
                            hs = halves_of(ki)
                            if hv < len(hs):
                                batch.append((ki, hs[hv]))
                        s_list = []
                        for ki, (h0, hl) in batch:
                            s_ps = psS.tile([128, NMAX], f32, tag="strip")
                            s_list.append(s_ps)
                        maxhl = max(hl for _, (_, hl) in batch)
                        off = 0
                        while off < maxhl:
                            for s_ps, (ki, (h0, hl)) in zip(s_list, batch):
                                if off >= hl:
                                    continue
                                n = min(512, hl - off)
                                kT, kb, qT, qb = srcs[ki]
                                nc.tensor.matmul(
                                    s_ps[:, off : off + n],
                                    kT[kb : kb + 64, ki * 128 : (ki + 1) * 128],
                                    qT[qb : qb + 64, h0 + off : h0 + off + n],
                                    start=True,
                                    stop=True,
                                    skip_group_check=True,
                                )
                            off += 512
                        if causal and hv == 0:
                            for s_ps in s_list:
                                nc.tensor.matmul(
                                    s_ps[:, 0:128], tri_tb[:], id_b[:],
                                    start=False, stop=True,
                                    skip_group_check=True,
                                )
                        for s_ps, (ki, (h0, hl)) in zip(s_list, batch):
                            et = e_p.tile([128, NMAX], bf16, tag="e")
                            acc = st_p.tile([128, 1], f32, tag="acc")
                            nc.scalar.activation(
                                et[:, 0:hl], s_ps[:, 0:hl], EXP,
                                scale=SCALE, accum_out=acc[:],
                            )
                            ets[ki].append((et, h0, hl))
                            accs[ki].append(acc)
                    if filler is not None:
                        filler()
                    return kis, ets, accs

                def do_av(kis, ets, accs):
                    for ki in kis:
                        vpt = make_vpt(accs[ki], ki, 2)
                        for et, h0, hl in ets[ki]:
                            # split chunks at the absolute-1024 boundary
                            a = h0
                            while a < h0 + hl:
                                lim = 1024 if a < 1024 else 2048
                                n = min(512, h0 + hl - a, lim - a)
                                phalf = 0 if a < 1024 else 64
                                c0 = a - (1024 if phalf else 0)
                                nc.tensor.matmul(
                                    av_ps[phalf : phalf + 64, c0 : c0 + n],
                                    vpt[:],
                                    et[:, a - h0 : a - h0 + n],
                                    start=(ki == 0),
                                    stop=(ki == 15),
                                    skip_group_check=True,
                                )
                                a += n

                pq = []  # 2-deep AV pipeline: chain latency ~2 t-steps
                for t in range(8):
                    fil = (lambda p=pq[0]: do_av(*p)) if len(pq) == 2 else None
                    cur = scores_batch(t, fil)
                    if fil is not None:
                        pq.pop(0)
                        pt_ = t - 2
                        if causal and pt_ % 2 == 1:
                            flushB(pt_ // 2)
                    if t % 2 == 0 and t < 6:
                        v_slot(t // 2 + 1, vnat)
                    pq.append(cur)
                for i, p in enumerate(pq):
                    do_av(*p)
                    pt_ = 6 + i
                    if causal and pt_ % 2 == 1:
                        flushB(pt_ // 2)
                if not causal:
                    for f in range(4):
                        flushB(f)

            # ---- wave A: heads 0+1 row/col paired, AV pipelined ----
            def waveA(av_ps, atn):
                def flushA(f):
                    cols = slice(512 * f, 512 * (f + 1))
                    nc.vector.tensor_copy(atn[:, cols], av_ps[:, cols])
                    nc.sync.dma_start(ag_inA[f][:, :], atn[:, cols])
                    nc.gpsimd.collective_compute(
                        "AllGather",
                        mybir.AluOpType.bypass,
                        replica_groups=groups,
                        ins=[ag_inA[f].ap().opt()],
                        outs=[ag_outA[f].ap().opt()],
                    )
                    for dt_i in range(4):
                        nc.sync.dma_start(
                            agA_t[f][dt_i][:, :],
                            ag_outA[f][dt_i * 128 : (dt_i + 1) * 128, :],
                        )

                def do_av(ki, hs, ets, accs):
                    vpts = [make_vpt(accs[hi], ki, hi) for hi in range(2)]
                    for hv, (h0, hl) in enumerate(hs):
                        off = 0
                        while off < hl:
                            n = min(512, hl - off)
                            for hi in range(2):
                                p_lo = 0 if hi == 0 else 64
                                et = ets[hi][hv][0]
                                nc.tensor.matmul(
                                    av_ps[p_lo : p_lo + 64, h0 + off : h0 + off + n],
                                    vpts[hi][:],
                                    et[:, off : off + n],
                                    start=(ki == 0),
                                    stop=(ki == 15),
                                    skip_group_check=True,
                                )
                            off += n

                def flushA3(half):
                    cols = slice(1536 + 256 * half, 1792 + 256 * half)
                    nc.vector.tensor_copy(atn[:, cols], av_ps[:, cols])
                    nc.sync.dma_start(ag_inA3h[half][:, :], atn[:, cols])
                    nc.gpsimd.collective_compute(
                        "AllGather",
                        mybir.AluOpType.bypass,
                        replica_groups=groups,
                        ins=[ag_inA3h[half].ap().opt()],
                        outs=[ag_outA3h[half].ap().opt()],
                    )
                    for dt_i in range(4):
                        nc.sync.dma_start(
                            agA_t[3][dt_i][:, 256 * half : 256 * half + 256],
                            ag_outA3h[half][dt_i * 128 : (dt_i + 1) * 128, :],
                        )

                oproj_at = {5: 0, 9: 1, 13: 2} if causal else {}
                pq = []  # 2-deep AV pipeline (pki = ki - 2)
                for ki in range(16):
                    hs = halves_of(ki)
                    ets = {0: [], 1: []}
                    accs = {0: [], 1: []}
                    done_av = len(pq) < 2
                    for hv, (h0, hl) in enumerate(hs):
                        # AV of ki-2 between this ki's strip batches:
                        # PE filler while the exps run
                        if hv == 1 and not done_av:
                            do_av(ki - 2, *pq.pop(0))
                            done_av = True
                        res = strip_pair(
                            ki,
                            [(k01, 0, q01, 0), (k01, 64, q01, 64)],
                            hv, h0, hl,
                        )
                        for hi, (et, acc) in enumerate(res):
                            ets[hi].append((et, h0, hl))
                            accs[hi].append(acc)
                    if not done_av:
                        do_av(ki - 2, *pq.pop(0))
                        done_av = True
                    if len(pq) == 1 and ki >= 2:
                        pki = ki - 2
                        if causal and pki % 4 == 3:
                            flushA(pki // 4)
                        if causal and pki == 13:
                            flushA3(0)
                        if pki in oproj_at:
                            outproj_chunk(oproj_at[pki])
                    pq.append((hs, ets, accs))
                for i, p in enumerate(pq):
                    pki = 14 + i
                    do_av(pki, *p)
                if causal:
                    flushA3(1)
                    outproj_chunk(3)
                else:
                    for f in range(4):
                        flushA(f)
                    for f in range(4):
                        outproj_chunk(f)

            # ---- run: waveB (with v slots 1..3 + v slot 0 upfront) ----
            psB_ctx = tc.tile_pool(name="psB", bufs=1, space="PSUM")
            psB = psB_ctx.__enter__()
            av2 = psB.tile([128, 1024], f32, tag="av2")
            atn2 = atn_p.tile([64, S], bf16, tag="atn2")
            v_slot(0, vnat)
            waveB(av2, atn2)
            psB_ctx.__exit__(None, None, None)
            xs_ctx.__exit__(None, None, None)
            xt_ctx.__exit__(None, None, None)

            # waveA uses a 2-buf strip pool (avA takes 4 PSUM banks)
            psS_ctx.__exit__(None, None, None)
            psS2_ctx = tc.tile_pool(name="psS2", bufs=2, space="PSUM")
            psS = psS2_ctx.__enter__()
            psA_ctx = tc.tile_pool(name="psA", bufs=1, space="PSUM")
            psA = psA_ctx.__enter__()
            avA = psA.tile([128, S], f32, tag="avA")
            atnA = atn_p.tile([128, S], bf16, tag="atnA")
            waveA(avA, atnA)
            psA_ctx.__exit__(None, None, None)
            psS2_ctx.__exit__(None, None, None)
    nc.compile()
    return nc


def _shards(x, mask, W_in, b_in, W_out, b_out):
    """Build per-core input maps (host-side sharding / layout prep)."""
    tri_np = np.where(
        np.arange(128)[None, :] < np.arange(128)[:, None], np.float32(NEG), 0.0
    ).astype(np.float32)
    # split-AllGather row order: rank pairs (h=3r,3r+1) then solos (h=3r+2)
    head_order = [0, 1, 3, 4, 6, 7, 9, 10, 2, 5, 8, 11]
    row_perm = np.concatenate([np.arange(h * 64, (h + 1) * 64) for h in head_order])
    in_maps = []
    for c in range(NCORES):
        b = c // GROUPS
        g = c % GROUPS
        hs = [3 * g, 3 * g + 1, 3 * g + 2]
        qc = [W_in[:, 64 * h : 64 * (h + 1)] for h in hs]
        kc = [W_in[:, D + 64 * h : D + 64 * (h + 1)] for h in hs]
        vc = W_in[:, 2 * D + 64 * hs[0] : 2 * D + 64 * (hs[2] + 1)]
        qb = [b_in[64 * h : 64 * (h + 1)] for h in hs]
        kb = [b_in[D + 64 * h : D + 64 * (h + 1)] for h in hs]
        vb = b_in[2 * D + 64 * hs[0] : 2 * D + 64 * (hs[2] + 1)]
        wqk = np.concatenate(
            [kc[0], kc[1], qc[0], qc[1], qc[2], kc[2]], axis=1
        ).astype(np.float32)
        bqk = np.concatenate([kb[0], kb[1], qb[0], qb[1], qb[2], kb[2]])
        in_maps.append(
            {
                "xT": np.ascontiguousarray(x[b].T, dtype=np.float32),
                "wqk": np.ascontiguousarray(wqk),
                "wv": np.ascontiguousarray(vc, dtype=np.float32),
                "bqkc": np.ascontiguousarray(bqk[:, None], dtype=np.float32),
                "bv4": np.ascontiguousarray(
                    np.tile(vb, 4)[None, :], dtype=np.float32
                ),
                "wout": np.ascontiguousarray(
                    W_out[row_perm, EPG * g : EPG * (g + 1)], dtype=np.float32
                ),
                "boutc": np.ascontiguousarray(
                    b_out[EPG * g : EPG * (g + 1), None], dtype=np.float32
                ),
                "tri": tri_np,
                "triT": np.ascontiguousarray(tri_np.T),
                "ident": np.eye(128, dtype=np.float32),
            }
        )
    return in_maps


def _numpy_ref(x, mask, W_in, b_in, W_out, b_out):
    qkv = x @ W_in + b_in
    q, k, v = np.split(qkv, 3, axis=2)
    q = q.reshape(B, S, H, DH).transpose(0, 2, 1, 3)
    k = k.reshape(B, S, H, DH).transpose(0, 2, 1, 3)
    v = v.reshape(B, S, H, DH).transpose(0, 2, 1, 3)
    attn = np.einsum("bhqd,bhkd->bhqk", q, k) / np.sqrt(np.float32(D))
    attn = np.where(mask == 0, -np.inf, attn)
    attn = attn - attn.max(axis=-2, keepdims=True)
    e = np.exp(attn)
    attn = e / e.sum(axis=-2, keepdims=True)
    out = np.einsum("bhqk,bhkd->bhqd", attn, v)
    out = out.transpose(0, 2, 1, 3).reshape(B, S, D)
    return (out @ W_out + b_out).astype(np.float32)


def _run(inputs, trace=False):
    from concourse.bass_utils import run_bass_kernel_spmd

    x = np.asarray(inputs["x"], dtype=np.float32)
    mask = np.asarray(inputs["mask"])
    W_in = np.asarray(inputs["W_in"], dtype=np.float32)
    b_in = np.asarray(inputs["b_in"], dtype=np.float32)
    W_out = np.asarray(inputs["W_out"], dtype=np.float32)
    b_out = np.asarray(inputs["b_out"], dtype=np.float32)

    m2 = np.asarray(mask).reshape(S, S)
    if np.array_equal(m2, np.tril(np.ones((S, S), m2.dtype))):
        causal = True
    elif np.array_equal(m2, np.ones((S, S), m2.dtype)):
        causal = False
    else:
        return _numpy_ref(x, mask, W_in, b_in, W_out, b_out), None

    key = ("nc", causal)
    if key not in _cache:
        _cache[key] = _build(causal)
    nc = _cache[key]

    in_maps = _shards(x, mask, W_in, b_in, W_out, b_out)
    res = run_bass_kernel_spmd(nc, in_maps, core_ids=list(range(NCORES)), trace=trace)

    full = np.empty((B, S, D), dtype=np.float32)
    for c in range(NCORES):
        b, g = c // GROUPS, c % GROUPS
        full[b, :, EPG * g : EPG * (g + 1)] = res.results[c]["out"].T
    return full, res


def kernel(**inputs) -> np.ndarray:
    out, _ = _run(inputs, trace=False)
    return out


# revision 28
# speedup vs baseline: 1.0412x; 1.0412x over previous
"""Trainium2 Bass kernel for nn_AttentionHead (B=2, S=2048, D=768, H=12).

Sharding: 8 cores = 2 batches x 4 head-groups (3 heads each).
Per core: QKV projection for its heads (transposed layout), causal
attention with softmax over the QUERY axis (reference peculiarity:
softmax dim=-2, scaled by sqrt(d_model)), AllGather of per-head outputs
within each batch's 4-core group, then a column-slice of the output
projection.  Host only slices / transposes / concatenates.

Layout / scheduling choices:
  - Scores are built transposed: S_T[k, q] so the softmax axis (q) is
    the SBUF free axis; ScalarE exp computes the row sums for free via
    accum_out.  The per-k normalizer is folded into V ("V'") so the
    attn @ v matmul consumes raw exp scores.
  - Causal structure (checked on host) skips ~40% of score blocks; the
    diagonal triangle is masked by a DVE add of -1e30 into PSUM.
  - x is DMA'd in S-halves and the QKV chunks are emitted dt-major so
    the projection matmuls pipeline with the DMA arrivals.
  - Waves are software-pipelined: AV(step-1) is emitted after
    scores(step), so the PE stays busy while ScalarE runs the exps
    (keeps the HAM clock at full rate).
  - waveB's AV accumulator is folded to [128, 1024] (q-halves in
    partition halves) freeing 2 PSUM banks -> 3 strip buffers.
  - Output projection is chunked per 512 columns and interleaved into
    waveA right after each chunk's AllGather lands.
"""

import contextlib
import math

import numpy as np

B, S, D, H, DH = 2, 2048, 768, 12, 64
NCORES = 8
GROUPS = 4  # head-groups per batch
HPG = 3  # heads per group
EPG = HPG * DH  # 192
SCALE = 1.0 / math.sqrt(D)
NEG = -1.0e30

_cache = {}


NMAX = 1024  # bf16 moving-operand max per matmul
SH = 1024  # S-half for x DMA staging


def _build(causal: bool):
    import concourse.bacc as bacc
    import concourse.mybir as mybir
    from concourse import tile

    f32 = mybir.dt.float32
    bf16 = mybir.dt.bfloat16
    EXP = mybir.ActivationFunctionType.Exp

    nc = bacc.Bacc("TRN2", target_bir_lowering=False, debug=False, num_devices=NCORES)

    xT = nc.dram_tensor("xT", [D, S], f32, kind="ExternalInput")
    wqk = nc.dram_tensor("wqk", [D, 384], f32, kind="ExternalInput")
    wv = nc.dram_tensor("wv", [D, EPG], f32, kind="ExternalInput")
    bqkc = nc.dram_tensor("bqkc", [384, 1], f32, kind="ExternalInput")
    bv4 = nc.dram_tensor("bv4", [1, 4 * EPG], f32, kind="ExternalInput")
    wout = nc.dram_tensor("wout", [D, EPG], f32, kind="ExternalInput")
    boutc = nc.dram_tensor("boutc", [EPG, 1], f32, kind="ExternalInput")
    tri = nc.dram_tensor("tri", [128, 128], f32, kind="ExternalInput")
    triT = nc.dram_tensor("triT", [128, 128], f32, kind="ExternalInput")
    ident = nc.dram_tensor("ident", [128, 128], f32, kind="ExternalInput")
    out = nc.dram_tensor("out", [EPG, S], f32, kind="ExternalOutput")

    ag_inA = [nc.dram_tensor(f"ag_inA{f}", [128, 512], bf16) for f in range(4)]
    ag_outA = [nc.dram_tensor(f"ag_outA{f}", [512, 512], bf16) for f in range(4)]
    # final flush split in two 256-col AllGathers to shorten the tail
    ag_inA3h = [nc.dram_tensor(f"ag_inA3h{i}", [128, 256], bf16) for i in range(2)]
    ag_outA3h = [nc.dram_tensor(f"ag_outA3h{i}", [512, 256], bf16) for i in range(2)]
    ag_inB = [nc.dram_tensor(f"ag_inB{f}", [64, 1024], bf16) for f in range(2)]
    ag_outB = [nc.dram_tensor(f"ag_outB{f}", [256, 1024], bf16) for f in range(2)]
    bar_in = nc.dram_tensor("bar_in", [1, 8], bf16)
    bar_out = nc.dram_tensor("bar_out", [4, 8], bf16)

    groups = [[0, 1, 2, 3], [4, 5, 6, 7]]

    with tile.TileContext(nc) as tc:
        with contextlib.ExitStack() as ctx:
            const_p = ctx.enter_context(tc.tile_pool(name="const", bufs=1))
            w_p = ctx.enter_context(tc.tile_pool(name="w", bufs=6))
            qk_p = ctx.enter_context(tc.tile_pool(name="qk", bufs=1))
            v_p = ctx.enter_context(tc.tile_pool(name="v", bufs=1))
            e_p = ctx.enter_context(tc.tile_pool(name="e", bufs=13))
            st_p = ctx.enter_context(tc.tile_pool(name="stat", bufs=24))
            vp_p = ctx.enter_context(tc.tile_pool(name="vp", bufs=10))
            atn_p = ctx.enter_context(tc.tile_pool(name="atn", bufs=1))
            ag_p = ctx.enter_context(tc.tile_pool(name="ag", bufs=1))
            o_p = ctx.enter_context(tc.tile_pool(name="o", bufs=2))

            psS_ctx = tc.tile_pool(name="psS3", bufs=3, space="PSUM")
            psS = psS_ctx.__enter__()

            # ---- constants (tiny DMAs first) ----
            ones_f = const_p.tile([1, 512], f32)
            nc.vector.memset(ones_f[:], 1.0)
            ones = const_p.tile([1, 512], bf16)
            nc.vector.tensor_copy(ones[:], ones_f[:])
            tri_f = const_p.tile([128, 128], f32)
            nc.sync.dma_start(tri_f[:], tri[:, :])
            triT_f = const_p.tile([128, 128], f32)
            nc.sync.dma_start(triT_f[:], triT[:, :])
            tri_tb = const_p.tile([128, 128], bf16)
            nc.vector.tensor_copy(tri_tb[:], triT_f[:])
            id_f = const_p.tile([128, 128], f32)
            nc.sync.dma_start(id_f[:], ident[:, :])
            id_b = const_p.tile([128, 128], bf16)
            nc.vector.tensor_copy(id_b[:], id_f[:])

            bqk_c = const_p.tile([128, 3], f32)
            nc.sync.dma_start(bqk_c[:], bqkc[:, :].rearrange("(c p) o -> p (c o)", p=128))
            bout_c = const_p.tile([128, 2], f32)
            nc.sync.dma_start(bout_c[0:64, 1:2], boutc[128:EPG, :])
            nc.sync.dma_start(bout_c[:, 0:1], boutc[0:128, :])
            bv_f = const_p.tile([1, 4 * EPG], f32)
            nc.sync.dma_start(bv_f[:], bv4[:, :])
            bv_t = const_p.tile([1, 4 * EPG], bf16)
            nc.vector.tensor_copy(bv_t[:], bv_f[:])

            # ---- barrier AllGather: absorb cross-core launch skew early,
            # while the PE is DMA-bound anyway, so the real AllGathers
            # later don't eat a ~25us first-collective peer wait ----
            bar_t = const_p.tile([1, 8], bf16)
            nc.vector.memset(bar_t[:], 0.0)
            nc.sync.dma_start(bar_in[:, :], bar_t[:])
            nc.gpsimd.collective_compute(
                "AllGather",
                mybir.AluOpType.bypass,
                replica_groups=groups,
                ins=[bar_in.ap().opt()],
                outs=[bar_out.ap().opt()],
            )

            # ---- PE warm-up: dummy matmuls while first DMAs land ----
            warm_in = const_p.tile([128, 512], bf16)
            nc.vector.memset(warm_in[:], 0.0)
            for wi in range(8):
                wps = psS.tile([128, NMAX], f32, tag="strip")
                nc.tensor.matmul(
                    wps[:, 0:512], id_b[:], warm_in[:],
                    start=True, stop=True, skip_group_check=True,
                )

            # ---- DMA priority: per-dt (wqk, x-sh0) interleaved so the
            # first qkv group paces with arrivals; then wv, x-sh1, wout ----
            xt_ctx = tc.tile_pool(name="xt", bufs=1)
            xt_p = xt_ctx.__enter__()
            xs_ctx = tc.tile_pool(name="xs", bufs=6)
            xs_p = xs_ctx.__enter__()
            xt_t = [xt_p.tile([128, S], bf16, tag=f"xt{i}", name=f"xt{i}") for i in range(6)]
            xs_t = [xs_p.tile([128, S], f32, tag="xstg", name=f"xs{i}") for i in range(6)]
            wqk_t, wv_t, wout_t = [], [], []
            c0 = slice(0, SH)
            c1 = slice(SH, S)
            for dt_i in range(6):
                wf = w_p.tile([128, 384], f32, tag="wstg")
                nc.sync.dma_start(wf[:], wqk[dt_i * 128 : (dt_i + 1) * 128, :])
                wt = w_p.tile([128, 384], bf16, tag="wqk")
                nc.vector.tensor_copy(wt[:], wf[:])
                wqk_t.append(wt)
                nc.sync.dma_start(
                    xs_t[dt_i][:, c0], xT[dt_i * 128 : (dt_i + 1) * 128, c0]
                )
                nc.vector.tensor_copy(xt_t[dt_i][:, c0], xs_t[dt_i][:, c0])
            for dt_i in range(6):
                vf = w_p.tile([128, EPG], f32, tag="wvstg")
                nc.sync.dma_start(vf[:], wv[dt_i * 128 : (dt_i + 1) * 128, :])
                vt = w_p.tile([128, EPG], bf16, tag="wv")
                nc.vector.tensor_copy(vt[:], vf[:])
                wv_t.append(vt)
            for dt_i in range(6):
                nc.sync.dma_start(
                    xs_t[dt_i][:, c1], xT[dt_i * 128 : (dt_i + 1) * 128, c1]
                )
                nc.vector.tensor_copy(xt_t[dt_i][:, c1], xs_t[dt_i][:, c1])

            # qkv chunk for one s-half of 1024; bias folded into the copy.
            # dt-major emission across a GROUP of chunks so the matmuls
            # pipeline with the per-dt x DMA arrivals.
            def qkv_group(specs, sc):
                """specs: list of (dst_tile, c). Emits all chunks of one
                s-half dt-major; each chunk uses its own PSUM slot."""
                pts = [psS.tile([128, NMAX], f32, tag="strip", name=f"qg{ci}") for ci in range(len(specs))]
                for dt_i in range(6):
                    for pt, (dst, c) in zip(pts, specs):
                        for off in range(0, NMAX, 512):
                            nc.tensor.matmul(
                                pt[:, off : off + 512],
                                wqk_t[dt_i][:, c * 128 : (c + 1) * 128],
                                xt_t[dt_i][:, sc * NMAX + off : sc * NMAX + off + 512],
                                start=(dt_i == 0), stop=(dt_i == 5),
                                skip_group_check=True,
                            )
                for pt, (dst, c) in zip(pts, specs):
                    nc.vector.tensor_scalar_add(
                        dst[:, sc * NMAX : (sc + 1) * NMAX], pt[:], bqk_c[:, c : c + 1]
                    )

            # packed v tiles: 4 s-blocks of 128 in one PSUM slot
            def v_slot(j, vnat):
                sts = range(4 * j, 4 * j + 4)
                ptf = psS.tile([128, NMAX], f32, tag="strip")
                p = ptf[:, 0 : 4 * EPG]
                for b0 in range(0, 4 * EPG, 512):
                    bn = min(512, 4 * EPG - b0)
                    nc.tensor.matmul(
                        ptf[:, b0 : b0 + bn], ones[:, 0:128],
                        bv_t[:, b0 : b0 + bn], start=True, stop=False,
                        skip_group_check=True,
                    )
                for dt_i in range(6):
                    for jj, st_i in enumerate(sts):
                        nc.tensor.matmul(
                            ptf[:, jj * EPG : (jj + 1) * EPG],
                            xt_t[dt_i][:, st_i * 128 : (st_i + 1) * 128],
                            wv_t[dt_i][:],
                            start=False, stop=(dt_i == 5),
                            skip_group_check=True,
                        )
                nc.vector.tensor_copy(
                    vnat[:, 4 * j * EPG : (4 * j + 4) * EPG], p
                )

            # ---- upfront: all qk chunks, dt-major per s-half ----
            k01 = qk_p.tile([128, S], bf16, tag="k01")
            q01 = qk_p.tile([128, S], bf16, tag="q01")
            qk2 = qk_p.tile([128, S], bf16, tag="qk2")
            k2 = qk_p.tile([64, S], bf16, tag="k2")
            q2d = qk_p.tile([128, S], bf16, tag="q2d")
            vnat = v_p.tile([128, 16 * EPG], bf16)
            qkv_group([(qk2, 2), (q01, 1), (k01, 0)], 0)
            qkv_group([(qk2, 2), (q01, 1), (k01, 0)], 1)
            nc.gpsimd.dma_start(k2[:], qk2[64:128, :])
            nc.gpsimd.dma_start(q2d[64:128, :], qk2[0:64, :])

            # wout lands last in the DMA queue; out-proj needs it much later
            for dt_i in range(6):
                wos = w_p.tile([128, EPG], f32, tag="wostg")
                nc.sync.dma_start(wos[:], wout[dt_i * 128 : (dt_i + 1) * 128, :])
                wo = w_p.tile([128, EPG], bf16, tag="wout")
                nc.gpsimd.tensor_copy(wo[:], wos[:])
                wout_t.append(wo)

            def halves_of(ki):
                q0 = 128 * ki if causal else 0
                L = S - q0
                hs = [(q0, min(L, NMAX))]
                if L > NMAX:
                    hs.append((q0 + NMAX, L - NMAX))
                return hs

            def strip_pair(ki, srcs, hv, h0, hl):
                """Emit the two row-group S matmuls adjacently, masks, exps.
                srcs: [(kT, kbase, qT, qbase), ...] for row groups lo/hi."""
                out_tiles = []
                s_list = []
                for (kT, kb, qT, qb) in srcs:
                    s_ps = psS.tile([128, NMAX], f32, tag="strip")
                    s_list.append(s_ps)
                off = 0
                while off < hl:
                    n = min(512, hl - off)
                    for s_ps, (kT, kb, qT, qb) in zip(s_list, srcs):
                        nc.tensor.matmul(
                            s_ps[:, off : off + n],
                            kT[kb : kb + 64, ki * 128 : (ki + 1) * 128],
                            qT[qb : qb + 64, h0 + off : h0 + off + n],
                            start=True,
                            stop=True,
                            skip_group_check=True,
                        )
                    off += n
                for s_ps in s_list:
                    if causal and hv == 0:
                        nc.vector.tensor_add(
                            s_ps[:, 0:128], s_ps[:, 0:128], tri_f[:]
                        )
                for s_ps in s_list:
                    et = e_p.tile([128, NMAX], bf16, tag="e")
                    acc = st_p.tile([128, 1], f32, tag="acc")
                    nc.scalar.activation(
                        et[:, 0:hl], s_ps[:, 0:hl], EXP,
                        scale=SCALE, accum_out=acc[:],
                    )
                    out_tiles.append((et, acc))
                return out_tiles

            def make_vpt(accs, ki, head):
                rcp = st_p.tile([128, 1], f32, tag="rcp")
                if len(accs) == 2:
                    ssum = st_p.tile([128, 1], f32, tag="ssum")
                    nc.vector.tensor_add(ssum[:], accs[0][:], accs[1][:])
                    nc.vector.reciprocal(rcp[:], ssum[:])
                else:
                    nc.vector.reciprocal(rcp[:], accs[0][:])
                vpt = vp_p.tile([128, 64], bf16, tag="vp")
                nc.vector.tensor_scalar_mul(
                    vpt[:],
                    vnat[:, ki * EPG + head * 64 : ki * EPG + (head + 1) * 64],
                    rcp[:],
                )
                return vpt

            agA_t = [[None] * 4 for _ in range(4)]  # [f][dt] -> [128, 512]
            for f in range(4):
                for dt_i in range(4):
                    agA_t[f][dt_i] = ag_p.tile(
                        [128, 512], bf16, tag=f"agA{f}_{dt_i}",
                        name=f"agA{f}_{dt_i}",
                    )
            agB_t = [[None] * 2 for _ in range(2)]  # [g][dt] -> [128, 1024]
            for g in range(2):
                for dt_i in range(2):
                    agB_t[g][dt_i] = ag_p.tile(
                        [128, 1024], bf16, tag=f"agB{g}_{dt_i}",
                        name=f"agB{g}_{dt_i}",
                    )

            # ---- output projection chunk: 512 cols, borrows a PSUM slot ----
            def outproj_chunk(f):
                g, part = f // 2, f % 2
                pt = psS.tile([128, NMAX], f32, tag="strip")
                for m0, mw, pc, bc in ((0, 128, 0, 0), (128, 64, 512, 1)):
                    for dt_i in range(4):
                        nc.tensor.matmul(
                            pt[0:mw, pc : pc + 512],
                            wout_t[dt_i][:, m0 : m0 + mw],
                            agA_t[f][dt_i][:, :],
                            start=(dt_i == 0), stop=False,
                            skip_group_check=True,
                        )
                    for di, dt_i in enumerate((4, 5)):
                        nc.tensor.matmul(
                            pt[0:mw, pc : pc + 512],
                            wout_t[dt_i][:, m0 : m0 + mw],
                            agB_t[g][dt_i - 4][:, 512 * part : 512 * part + 512],
                            start=False, stop=(di == 1),
                            skip_group_check=True,
                        )
                    ot = o_p.tile([128, 512], f32, tag=f"oc{bc}")
                    nc.vector.tensor_scalar_add(
                        ot[0:mw, :], pt[0:mw, pc : pc + 512], bout_c[0:mw, bc : bc + 1]
                    )
                    nc.sync.dma_start(
                        out[m0 : m0 + mw, 512 * f : 512 * (f + 1)], ot[0:mw, :]
                    )

            # ---- wave B: head 2, ki pairs in row groups; AV pipelined ----
            # av2 folded: q cols [0,1024) in partitions 0:64,
            #             q cols [1024,2048) in partitions 64:128.
            def waveB(av_ps, atn):
                def flushB(f):
                    phalf = 64 * (f // 2)
                    c0 = 512 * (f % 2)
                    nc.vector.tensor_copy(
                        atn[:, 512 * f : 512 * (f + 1)],
                        av_ps[phalf : phalf + 64, c0 : c0 + 512],
                    )
                    if f % 2 == 0:
                        return
                    g = f // 2
                    gcols = slice(1024 * g, 1024 * (g + 1))
                    nc.sync.dma_start(ag_inB[g][:, :], atn[:, gcols])
                    nc.gpsimd.collective_compute(
                        "AllGather",
                        mybir.AluOpType.bypass,
                        replica_groups=groups,
                        ins=[ag_inB[g].ap().opt()],
                        outs=[ag_outB[g].ap().opt()],
                    )
                    for dt_i in range(2):
                        nc.sync.dma_start(
                            agB_t[g][dt_i][:, :],
                            ag_outB[g][dt_i * 128 : (dt_i + 1) * 128, :],
                        )

                def scores_batch(t, filler=None):
                    kis = (2 * t, 2 * t + 1)
                    srcs = {
                        kis[0]: (k2, 0, qk2, 0),
                        kis[1]: (qk2, 64, q2d, 64),
                    }
                    ets = {ki: [] for ki in kis}
                    accs = {ki: [] for ki in kis}
                    maxhv = max(len(halves_of(ki)) for ki in kis)
                    for hv in range(maxhv):
                        # PE filler between strip batches so the exps of the
                        # previous batch overlap matmul work
                        if hv == 1 and filler is not None:
                            filler()
                            filler = None
                        batch = []
                        for ki in kis:
                            hs = halves_of(ki)
                            if hv < len(hs):
                                batch.append((ki, hs[hv]))
                        s_list = []
                        for ki, (h0, hl) in batch:
                            s_ps = psS.tile([128, NMAX], f32, tag="strip")
                            s_list.append(s_ps)
                        maxhl = max(hl for _, (_, hl) in batch)
                        off = 0
                        while off < maxhl:
                            for s_ps, (ki, (h0, hl)) in zip(s_list, batch):
                                if off >= hl:
                                    continue
                                n = min(512, hl - off)
                                kT, kb, qT, qb = srcs[ki]
                                nc.tensor.matmul(
                                    s_ps[:, off : off + n],
                                    kT[kb : kb + 64, ki * 128 : (ki + 1) * 128],
                                    qT[qb : qb + 64, h0 + off : h0 + off + n],
                                    start=True,
                                    stop=True,
                                    skip_group_check=True,
                                )
                            off += 512
                        for s_ps, (ki, (h0, hl)) in zip(s_list, batch):
                            if causal and hv == 0:
                                nc.vector.tensor_add(
                                    s_ps[:, 0:128], s_ps[:, 0:128], tri_f[:]
                                )
                        for s_ps, (ki, (h0, hl)) in zip(s_list, batch):
                            et = e_p.tile([128, NMAX], bf16, tag="e")
                            acc = st_p.tile([128, 1], f32, tag="acc")
                            nc.scalar.activation(
                                et[:, 0:hl], s_ps[:, 0:hl], EXP,
                                scale=SCALE, accum_out=acc[:],
                            )
                            ets[ki].append((et, h0, hl))
                            accs[ki].append(acc)
                    if filler is not None:
                        filler()
                    return kis, ets, accs

                def do_av(kis, ets, accs):
                    for ki in kis:
                        vpt = make_vpt(accs[ki], ki, 2)
                        for et, h0, hl in ets[ki]:
                            # split chunks at the absolute-1024 boundary
                            a = h0
                            while a < h0 + hl:
                                lim = 1024 if a < 1024 else 2048
                                n = min(512, h0 + hl - a, lim - a)
                                phalf = 0 if a < 1024 else 64
                                c0 = a - (1024 if phalf else 0)
                                nc.tensor.matmul(
                                    av_ps[phalf : phalf + 64, c0 : c0 + n],
                                    vpt[:],
                                    et[:, a - h0 : a - h0 + n],
                                    start=(ki == 0),
                                    stop=(ki == 15),
                                    skip_group_check=True,
                                )
                                a += n

                pq = []  # 2-deep AV pipeline: chain latency ~2 t-steps
                for t in range(8):
                    fil = (lambda p=pq[0]: do_av(*p)) if len(pq) == 2 else None
                    cur = scores_batch(t, fil)
                    if fil is not None:
                        pq.pop(0)
                        pt_ = t - 2
                        if causal and pt_ % 2 == 1:
                            flushB(pt_ // 2)
                    if t % 2 == 0 and t < 6:
                        v_slot(t // 2 + 1, vnat)
                    pq.append(cur)
                for i, p in enumerate(pq):
                    do_av(*p)
                    pt_ = 6 + i
                    if causal and pt_ % 2 == 1:
                        flushB(pt_ // 2)
                if not causal:
                    for f in range(4):
                        flushB(f)

            # ---- wave A: heads 0+1 row/col paired, AV pipelined ----
            def waveA(av_ps, atn):
                def flushA(f):
                    cols = slice(512 * f, 512 * (f + 1))
                    nc.vector.tensor_copy(atn[:, cols], av_ps[:, cols])
                    nc.sync.dma_start(ag_inA[f][:, :], atn[:, cols])
                    nc.gpsimd.collective_compute(
                        "AllGather",
                        mybir.AluOpType.bypass,
                        replica_groups=groups,
                        ins=[ag_inA[f].ap().opt()],
                        outs=[ag_outA[f].ap().opt()],
                    )
                    for dt_i in range(4):
                        nc.sync.dma_start(
                            agA_t[f][dt_i][:, :],
                            ag_outA[f][dt_i * 128 : (dt_i + 1) * 128, :],
                        )

                def do_av(ki, hs, ets, accs):
                    vpts = [make_vpt(accs[hi], ki, hi) for hi in range(2)]
                    for hv, (h0, hl) in enumerate(hs):
                        off = 0
                        while off < hl:
                            n = min(512, hl - off)
                            for hi in range(2):
                                p_lo = 0 if hi == 0 else 64
                                et = ets[hi][hv][0]
                                nc.tensor.matmul(
                                    av_ps[p_lo : p_lo + 64, h0 + off : h0 + off + n],
                                    vpts[hi][:],
                                    et[:, off : off + n],
                                    start=(ki == 0),
                                    stop=(ki == 15),
                                    skip_group_check=True,
                                )
                            off += n

                def flushA3(half):
                    cols = slice(1536 + 256 * half, 1792 + 256 * half)
                    nc.vector.tensor_copy(atn[:, cols], av_ps[:, cols])
                    nc.sync.dma_start(ag_inA3h[half][:, :], atn[:, cols])
                    nc.gpsimd.collective_compute(
                        "AllGather",
                        mybir.AluOpType.bypass,
                        replica_groups=groups,
                        ins=[ag_inA3h[half].ap().opt()],
                        outs=[ag_outA3h[half].ap().opt()],
                    )
                    for dt_i in range(4):
                        nc.sync.dma_start(
                            agA_t[3][dt_i][:, 256 * half : 256 * half + 256],
                            ag_outA3h[half][dt_i * 128 : (dt_i + 1) * 128, :],
                        )

                oproj_at = {5: 0, 9: 1, 13: 2} if causal else {}
                pq = []  # 2-deep AV pipeline (pki = ki - 2)
                for ki in range(16):
                    hs = halves_of(ki)
                    ets = {0: [], 1: []}
                    accs = {0: [], 1: []}
                    done_av = len(pq) < 2
                    for hv, (h0, hl) in enumerate(hs):
                        # AV of ki-2 between this ki's strip batches:
                        # PE filler while the exps run
                        if hv == 1 and not done_av:
                            do_av(ki - 2, *pq.pop(0))
                            done_av = True
                        res = strip_pair(
                            ki,
                            [(k01, 0, q01, 0), (k01, 64, q01, 64)],
                            hv, h0, hl,
                        )
                        for hi, (et, acc) in enumerate(res):
                            ets[hi].append((et, h0, hl))
                            accs[hi].append(acc)
                    if not done_av:
                        do_av(ki - 2, *pq.pop(0))
                        done_av = True
                    if len(pq) == 1 and ki >= 2:
                        pki = ki - 2
                        if causal and pki % 4 == 3:
                            flushA(pki // 4)
                        if causal and pki == 13:
                            flushA3(0)
                        if pki in oproj_at:
                            outproj_chunk(oproj_at[pki])
                    pq.append((hs, ets, accs))
                for i, p in enumerate(pq):
                    pki = 14 + i
                    do_av(pki, *p)
                if causal:
                    flushA3(1)
                    outproj_chunk(3)
                else:
                    for f in range(4):
                        flushA(f)
                    for f in range(4):
                        outproj_chunk(f)

            # ---- run: waveB (with v slots 1..3 + v slot 0 upfront) ----
            psB_ctx = tc.tile_pool(name="psB", bufs=1, space="PSUM")
            psB = psB_ctx.__enter__()
            av2 = psB.tile([128, 1024], f32, tag="av2")
            atn2 = atn_p.tile([64, S], bf16, tag="atn2")
            v_slot(0, vnat)
            waveB(av2, atn2)
            psB_ctx.__exit__(None, None, None)
            xs_ctx.__exit__(None, None, None)
            xt_ctx.__exit__(None, None, None)

            # waveA uses a 2-buf strip pool (avA takes 4 PSUM banks)
            psS_ctx.__exit__(None, None, None)
            psS2_ctx = tc.tile_pool(name="psS2", bufs=2, space="PSUM")
            psS = psS2_ctx.__enter__()
            psA_ctx = tc.tile_pool(name="psA", bufs=1, space="PSUM")
            psA = psA_ctx.__enter__()
            avA = psA.tile([128, S], f32, tag="avA")
            atnA = atn_p.tile([128, S], bf16, tag="atnA")
            waveA(avA, atnA)
            psA_ctx.__exit__(None, None, None)
            psS2_ctx.__exit__(None, None, None)
    nc.compile()
    return nc


def _shards(x, mask, W_in, b_in, W_out, b_out):
    """Build per-core input maps (host-side sharding / layout prep)."""
    tri_np = np.where(
        np.arange(128)[None, :] < np.arange(128)[:, None], np.float32(NEG), 0.0
    ).astype(np.float32)
    # split-AllGather row order: rank pairs (h=3r,3r+1) then solos (h=3r+2)
    head_order = [0, 1, 3, 4, 6, 7, 9, 10, 2, 5, 8, 11]
    row_perm = np.concatenate([np.arange(h * 64, (h + 1) * 64) for h in head_order])
    in_maps = []
    for c in range(NCORES):
        b = c // GROUPS
        g = c % GROUPS
        hs = [3 * g, 3 * g + 1, 3 * g + 2]
        qc = [W_in[:, 64 * h : 64 * (h + 1)] for h in hs]
        kc = [W_in[:, D + 64 * h : D + 64 * (h + 1)] for h in hs]
        vc = W_in[:, 2 * D + 64 * hs[0] : 2 * D + 64 * (hs[2] + 1)]
        qb = [b_in[64 * h : 64 * (h + 1)] for h in hs]
        kb = [b_in[D + 64 * h : D + 64 * (h + 1)] for h in hs]
        vb = b_in[2 * D + 64 * hs[0] : 2 * D + 64 * (hs[2] + 1)]
        wqk = np.concatenate(
            [kc[0], kc[1], qc[0], qc[1], qc[2], kc[2]], axis=1
        ).astype(np.float32)
        bqk = np.concatenate([kb[0], kb[1], qb[0], qb[1], qb[2], kb[2]])
        in_maps.append(
            {
                "xT": np.ascontiguousarray(x[b].T, dtype=np.float32),
                "wqk": np.ascontiguousarray(wqk),
                "wv": np.ascontiguousarray(vc, dtype=np.float32),
                "bqkc": np.ascontiguousarray(bqk[:, None], dtype=np.float32),
                "bv4": np.ascontiguousarray(
                    np.tile(vb, 4)[None, :], dtype=np.float32
                ),
                "wout": np.ascontiguousarray(
                    W_out[row_perm, EPG * g : EPG * (g + 1)], dtype=np.float32
                ),
                "boutc": np.ascontiguousarray(
                    b_out[EPG * g : EPG * (g + 1), None], dtype=np.float32
                ),
                "tri": tri_np,
                "triT": np.ascontiguousarray(tri_np.T),
                "ident": np.eye(128, dtype=np.float32),
            }
        )
    return in_maps


def _numpy_ref(x, mask, W_in, b_in, W_out, b_out):
    qkv = x @ W_in + b_in
    q, k, v = np.split(qkv, 3, axis=2)
    q = q.reshape(B, S, H, DH).transpose(0, 2, 1, 3)
    k = k.reshape(B, S, H, DH).transpose(0, 2, 1, 3)
    v = v.reshape(B, S, H, DH).transpose(0, 2, 1, 3)
    attn = np.einsum("bhqd,bhkd->bhqk", q, k) / np.sqrt(np.float32(D))
    attn = np.where(mask == 0, -np.inf, attn)
    attn = attn - attn.max(axis=-2, keepdims=True)
    e = np.exp(attn)
    attn = e / e.sum(axis=-2, keepdims=True)
    out = np.einsum("bhqk,bhkd->bhqd", attn, v)
    out = out.transpose(0, 2, 1, 3).reshape(B, S, D)
    return (out @ W_out + b_out).astype(np.float32)


def _run(inputs, trace=False):
    from concourse.bass_utils import run_bass_kernel_spmd

    x = np.asarray(inputs["x"], dtype=np.float32)
    mask = np.asarray(inputs["mask"])
    W_in = np.asarray(inputs["W_in"], dtype=np.float32)
    b_in = np.asarray(inputs["b_in"], dtype=np.float32)
    W_out = np.asarray(inputs["W_out"], dtype=np.float32)
    b_out = np.asarray(inputs["b_out"], dtype=np.float32)

    m2 = np.asarray(mask).reshape(S, S)
    if np.array_equal(m2, np.tril(np.ones((S, S), m2.dtype))):
        causal = True
    elif np.array_equal(m2, np.ones((S, S), m2.dtype)):
        causal = False
    else:
        return _numpy_ref(x, mask, W_in, b_in, W_out, b_out), None

    key = ("nc", causal)
    if key not in _cache:
        _cache[key] = _build(causal)
    nc = _cache[key]

    in_maps = _shards(x, mask, W_in, b_in, W_out, b_out)
    res = run_bass_kernel_spmd(nc, in_maps, core_ids=list(range(NCORES)), trace=trace)

    full = np.empty((B, S, D), dtype=np.float32)
    for c in range(NCORES):
        b, g = c // GROUPS, c % GROUPS
        full[b, :, EPG * g : EPG * (g + 1)] = res.results[c]["out"].T
    return full, res


def kernel(**inputs) -> np.ndarray:
    out, _ = _run(inputs, trace=False)
    return out


# revision 30
# speedup vs baseline: 1.0582x; 1.0163x over previous
"""Trainium2 Bass kernel for nn_AttentionHead (B=2, S=2048, D=768, H=12).

Sharding: 8 cores = 2 batches x 4 head-groups (3 heads each).
Per core: QKV projection for its heads (transposed layout), causal
attention with softmax over the QUERY axis (reference peculiarity:
softmax dim=-2, scaled by sqrt(d_model)), AllGather of per-head outputs
within each batch's 4-core group, then a column-slice of the output
projection.  Host only slices / transposes / concatenates.

Layout / scheduling choices:
  - Scores are built transposed: S_T[k, q] so the softmax axis (q) is
    the SBUF free axis; ScalarE exp computes the row sums for free via
    accum_out.  The per-k normalizer is folded into V ("V'") so the
    attn @ v matmul consumes raw exp scores.
  - Causal structure (checked on host) skips ~40% of score blocks; the
    diagonal triangle is masked by a DVE add of -1e30 into PSUM.
  - x is DMA'd in S-halves and the QKV chunks are emitted dt-major so
    the projection matmuls pipeline with the DMA arrivals.
  - Waves are software-pipelined: AV(step-1) is emitted after
    scores(step), so the PE stays busy while ScalarE runs the exps
    (keeps the HAM clock at full rate).
  - waveB's AV accumulator is folded to [128, 1024] (q-halves in
    partition halves) freeing 2 PSUM banks -> 3 strip buffers.
  - Output projection is chunked per 512 columns and interleaved into
    waveA right after each chunk's AllGather lands.
"""

import contextlib
import math

import numpy as np

B, S, D, H, DH = 2, 2048, 768, 12, 64
NCORES = 8
GROUPS = 4  # head-groups per batch
HPG = 3  # heads per group
EPG = HPG * DH  # 192
SCALE = 1.0 / math.sqrt(D)
NEG = -1.0e30

_cache = {}


NMAX = 1024  # bf16 moving-operand max per matmul
SH = 1024  # S-half for x DMA staging


def _build(causal: bool):
    import concourse.bacc as bacc
    import concourse.mybir as mybir
    from concourse import tile

    f32 = mybir.dt.float32
    bf16 = mybir.dt.bfloat16
    EXP = mybir.ActivationFunctionType.Exp

    nc = bacc.Bacc("TRN2", target_bir_lowering=False, debug=False, num_devices=NCORES)

    xT = nc.dram_tensor("xT", [D, S], f32, kind="ExternalInput")
    wqk = nc.dram_tensor("wqk", [D, 384], f32, kind="ExternalInput")
    wv = nc.dram_tensor("wv", [D, EPG], f32, kind="ExternalInput")
    bqkc = nc.dram_tensor("bqkc", [384, 1], f32, kind="ExternalInput")
    bv4 = nc.dram_tensor("bv4", [1, 4 * EPG], f32, kind="ExternalInput")
    wout = nc.dram_tensor("wout", [D, EPG], f32, kind="ExternalInput")
    boutc = nc.dram_tensor("boutc", [EPG, 1], f32, kind="ExternalInput")
    tri = nc.dram_tensor("tri", [128, 128], f32, kind="ExternalInput")
    triT = nc.dram_tensor("triT", [128, 128], f32, kind="ExternalInput")
    ident = nc.dram_tensor("ident", [128, 128], f32, kind="ExternalInput")
    out = nc.dram_tensor("out", [EPG, S], f32, kind="ExternalOutput")

    ag_inA = [nc.dram_tensor(f"ag_inA{f}", [128, 512], bf16) for f in range(4)]
    ag_outA = [nc.dram_tensor(f"ag_outA{f}", [512, 512], bf16) for f in range(4)]
    # final flush split in two 256-col AllGathers to shorten the tail
    ag_inA3h = [nc.dram_tensor(f"ag_inA3h{i}", [128, 256], bf16) for i in range(2)]
    ag_outA3h = [nc.dram_tensor(f"ag_outA3h{i}", [512, 256], bf16) for i in range(2)]
    ag_inB = [nc.dram_tensor(f"ag_inB{f}", [64, 1024], bf16) for f in range(2)]
    ag_outB = [nc.dram_tensor(f"ag_outB{f}", [256, 1024], bf16) for f in range(2)]
    bar_in = nc.dram_tensor("bar_in", [1, 8], bf16)
    bar_out = nc.dram_tensor("bar_out", [4, 8], bf16)

    groups = [[0, 1, 2, 3], [4, 5, 6, 7]]

    with tile.TileContext(nc) as tc:
        with contextlib.ExitStack() as ctx:
            const_p = ctx.enter_context(tc.tile_pool(name="const", bufs=1))
            w_p = ctx.enter_context(tc.tile_pool(name="w", bufs=6))
            qk_p = ctx.enter_context(tc.tile_pool(name="qk", bufs=1))
            v_p = ctx.enter_context(tc.tile_pool(name="v", bufs=1))
            e_p = ctx.enter_context(tc.tile_pool(name="e", bufs=13))
            st_p = ctx.enter_context(tc.tile_pool(name="stat", bufs=24))
            vp_p = ctx.enter_context(tc.tile_pool(name="vp", bufs=10))
            atn_p = ctx.enter_context(tc.tile_pool(name="atn", bufs=1))
            ag_p = ctx.enter_context(tc.tile_pool(name="ag", bufs=1))
            o_p = ctx.enter_context(tc.tile_pool(name="o", bufs=2))

            psS_ctx = tc.tile_pool(name="psS3", bufs=3, space="PSUM")
            psS = psS_ctx.__enter__()

            # strip allocation rotates in an extra 1-buf pool once the low
            # half of waveA's accumulator is flushed and its banks freed
            strip_state = {"extra": None, "cnt": 0}

            def strip_alloc(nm="sp"):
                strip_state["cnt"] += 1
                if strip_state["extra"] is not None and strip_state["cnt"] % 3 == 0:
                    return strip_state["extra"].tile(
                        [128, NMAX], f32, tag="strip", name=nm
                    )
                return psS.tile([128, NMAX], f32, tag="strip", name=nm)

            # ---- constants (tiny DMAs first) ----
            ones_f = const_p.tile([1, 512], f32)
            nc.vector.memset(ones_f[:], 1.0)
            ones = const_p.tile([1, 512], bf16)
            nc.vector.tensor_copy(ones[:], ones_f[:])
            tri_f = const_p.tile([128, 128], f32)
            nc.sync.dma_start(tri_f[:], tri[:, :])
            triT_f = const_p.tile([128, 128], f32)
            nc.sync.dma_start(triT_f[:], triT[:, :])
            tri_tb = const_p.tile([128, 128], bf16)
            nc.vector.tensor_copy(tri_tb[:], triT_f[:])
            id_f = const_p.tile([128, 128], f32)
            nc.sync.dma_start(id_f[:], ident[:, :])
            id_b = const_p.tile([128, 128], bf16)
            nc.vector.tensor_copy(id_b[:], id_f[:])

            bqk_c = const_p.tile([128, 3], f32)
            nc.sync.dma_start(bqk_c[:], bqkc[:, :].rearrange("(c p) o -> p (c o)", p=128))
            bout_c = const_p.tile([128, 2], f32)
            nc.sync.dma_start(bout_c[0:64, 1:2], boutc[128:EPG, :])
            nc.sync.dma_start(bout_c[:, 0:1], boutc[0:128, :])
            bv_f = const_p.tile([1, 4 * EPG], f32)
            nc.sync.dma_start(bv_f[:], bv4[:, :])
            bv_t = const_p.tile([1, 4 * EPG], bf16)
            nc.vector.tensor_copy(bv_t[:], bv_f[:])

            # ---- barrier AllGather: absorb cross-core launch skew early,
            # while the PE is DMA-bound anyway, so the real AllGathers
            # later don't eat a ~25us first-collective peer wait ----
            bar_t = const_p.tile([1, 8], bf16)
            nc.vector.memset(bar_t[:], 0.0)
            nc.sync.dma_start(bar_in[:, :], bar_t[:])
            nc.gpsimd.collective_compute(
                "AllGather",
                mybir.AluOpType.bypass,
                replica_groups=groups,
                ins=[bar_in.ap().opt()],
                outs=[bar_out.ap().opt()],
            )

            # ---- PE warm-up: dummy matmuls while first DMAs land ----
            warm_in = const_p.tile([128, 512], bf16)
            nc.vector.memset(warm_in[:], 0.0)
            for wi in range(8):
                wps = strip_alloc("wps")
                nc.tensor.matmul(
                    wps[:, 0:512], id_b[:], warm_in[:],
                    start=True, stop=True, skip_group_check=True,
                )

            # ---- DMA priority: per-dt (wqk, x-sh0) interleaved so the
            # first qkv group paces with arrivals; then wv, x-sh1, wout ----
            xt_ctx = tc.tile_pool(name="xt", bufs=1)
            xt_p = xt_ctx.__enter__()
            xs_ctx = tc.tile_pool(name="xs", bufs=6)
            xs_p = xs_ctx.__enter__()
            xt_t = [xt_p.tile([128, S], bf16, tag=f"xt{i}", name=f"xt{i}") for i in range(6)]
            xs_t = [xs_p.tile([128, S], f32, tag="xstg", name=f"xs{i}") for i in range(6)]
            wqk_t, wv_t, wout_t = [], [], []
            c0 = slice(0, SH)
            c1 = slice(SH, S)
            for dt_i in range(6):
                wf = w_p.tile([128, 384], f32, tag="wstg")
                nc.sync.dma_start(wf[:], wqk[dt_i * 128 : (dt_i + 1) * 128, :])
                wt = w_p.tile([128, 384], bf16, tag="wqk")
                nc.vector.tensor_copy(wt[:], wf[:])
                wqk_t.append(wt)
                nc.sync.dma_start(
                    xs_t[dt_i][:, c0], xT[dt_i * 128 : (dt_i + 1) * 128, c0]
                )
                nc.vector.tensor_copy(xt_t[dt_i][:, c0], xs_t[dt_i][:, c0])
            for dt_i in range(6):
                vf = w_p.tile([128, EPG], f32, tag="wvstg")
                nc.sync.dma_start(vf[:], wv[dt_i * 128 : (dt_i + 1) * 128, :])
                vt = w_p.tile([128, EPG], bf16, tag="wv")
                nc.vector.tensor_copy(vt[:], vf[:])
                wv_t.append(vt)
            for dt_i in range(6):
                nc.sync.dma_start(
                    xs_t[dt_i][:, c1], xT[dt_i * 128 : (dt_i + 1) * 128, c1]
                )
                nc.vector.tensor_copy(xt_t[dt_i][:, c1], xs_t[dt_i][:, c1])

            # qkv chunk for one s-half of 1024; bias folded into the copy.
            # dt-major emission across a GROUP of chunks so the matmuls
            # pipeline with the per-dt x DMA arrivals.
            def qkv_group(specs, sc):
                """specs: list of (dst_tile, c). Emits all chunks of one
                s-half dt-major; each chunk uses its own PSUM slot."""
                pts = [strip_alloc(f"qg{ci}") for ci in range(len(specs))]
                for dt_i in range(6):
                    for pt, (dst, c) in zip(pts, specs):
                        for off in range(0, NMAX, 512):
                            nc.tensor.matmul(
                                pt[:, off : off + 512],
                                wqk_t[dt_i][:, c * 128 : (c + 1) * 128],
                                xt_t[dt_i][:, sc * NMAX + off : sc * NMAX + off + 512],
                                start=(dt_i == 0), stop=(dt_i == 5),
                                skip_group_check=True,
                            )
                for pt, (dst, c) in zip(pts, specs):
                    nc.vector.tensor_scalar_add(
                        dst[:, sc * NMAX : (sc + 1) * NMAX], pt[:], bqk_c[:, c : c + 1]
                    )

            # packed v tiles: 4 s-blocks of 128 in one PSUM slot
            def v_slot(j, vnat):
                sts = range(4 * j, 4 * j + 4)
                ptf = strip_alloc("ptf")
                p = ptf[:, 0 : 4 * EPG]
                for b0 in range(0, 4 * EPG, 512):
                    bn = min(512, 4 * EPG - b0)
                    nc.tensor.matmul(
                        ptf[:, b0 : b0 + bn], ones[:, 0:128],
                        bv_t[:, b0 : b0 + bn], start=True, stop=False,
                        skip_group_check=True,
                    )
                for dt_i in range(6):
                    for jj, st_i in enumerate(sts):
                        nc.tensor.matmul(
                            ptf[:, jj * EPG : (jj + 1) * EPG],
                            xt_t[dt_i][:, st_i * 128 : (st_i + 1) * 128],
                            wv_t[dt_i][:],
                            start=False, stop=(dt_i == 5),
                            skip_group_check=True,
                        )
                nc.vector.tensor_copy(
                    vnat[:, 4 * j * EPG : (4 * j + 4) * EPG], p
                )

            # ---- upfront: all qk chunks, dt-major per s-half ----
            k01 = qk_p.tile([128, S], bf16, tag="k01")
            q01 = qk_p.tile([128, S], bf16, tag="q01")
            qk2 = qk_p.tile([128, S], bf16, tag="qk2")
            k2 = qk_p.tile([64, S], bf16, tag="k2")
            q2d = qk_p.tile([128, S], bf16, tag="q2d")
            vnat = v_p.tile([128, 16 * EPG], bf16)
            qkv_group([(qk2, 2), (q01, 1), (k01, 0)], 0)
            qkv_group([(qk2, 2), (q01, 1), (k01, 0)], 1)
            nc.gpsimd.dma_start(k2[:], qk2[64:128, :])
            nc.gpsimd.dma_start(q2d[64:128, :], qk2[0:64, :])

            # wout lands last in the DMA queue; out-proj needs it much later
            for dt_i in range(6):
                wos = w_p.tile([128, EPG], f32, tag="wostg")
                nc.sync.dma_start(wos[:], wout[dt_i * 128 : (dt_i + 1) * 128, :])
                wo = w_p.tile([128, EPG], bf16, tag="wout")
                nc.gpsimd.tensor_copy(wo[:], wos[:])
                wout_t.append(wo)

            def halves_of(ki):
                q0 = 128 * ki if causal else 0
                L = S - q0
                hs = [(q0, min(L, NMAX))]
                if L > NMAX:
                    hs.append((q0 + NMAX, L - NMAX))
                return hs

            def strip_pair(ki, srcs, hv, h0, hl):
                """Emit the two row-group S matmuls adjacently, masks, exps.
                srcs: [(kT, kbase, qT, qbase), ...] for row groups lo/hi."""
                out_tiles = []
                s_list = []
                for (kT, kb, qT, qb) in srcs:
                    s_ps = strip_alloc("spa")
                    s_list.append(s_ps)
                off = 0
                while off < hl:
                    n = min(512, hl - off)
                    for s_ps, (kT, kb, qT, qb) in zip(s_list, srcs):
                        nc.tensor.matmul(
                            s_ps[:, off : off + n],
                            kT[kb : kb + 64, ki * 128 : (ki + 1) * 128],
                            qT[qb : qb + 64, h0 + off : h0 + off + n],
                            start=True,
                            stop=True,
                            skip_group_check=True,
                        )
                    off += n
                for s_ps in s_list:
                    if causal and hv == 0:
                        nc.vector.tensor_add(
                            s_ps[:, 0:128], s_ps[:, 0:128], tri_f[:]
                        )
                for s_ps in s_list:
                    et = e_p.tile([128, NMAX], bf16, tag="e")
                    acc = st_p.tile([128, 1], f32, tag="acc")
                    nc.scalar.activation(
                        et[:, 0:hl], s_ps[:, 0:hl], EXP,
                        scale=SCALE, accum_out=acc[:],
                    )
                    out_tiles.append((et, acc))
                return out_tiles

            def make_vpt(accs, ki, head):
                rcp = st_p.tile([128, 1], f32, tag="rcp")
                if len(accs) == 2:
                    ssum = st_p.tile([128, 1], f32, tag="ssum")
                    nc.vector.tensor_add(ssum[:], accs[0][:], accs[1][:])
                    nc.vector.reciprocal(rcp[:], ssum[:])
                else:
                    nc.vector.reciprocal(rcp[:], accs[0][:])
                vpt = vp_p.tile([128, 64], bf16, tag="vp")
                nc.vector.tensor_scalar_mul(
                    vpt[:],
                    vnat[:, ki * EPG + head * 64 : ki * EPG + (head + 1) * 64],
                    rcp[:],
                )
                return vpt

            agA_t = [[None] * 4 for _ in range(4)]  # [f][dt] -> [128, 512]
            for f in range(4):
                for dt_i in range(4):
                    agA_t[f][dt_i] = ag_p.tile(
                        [128, 512], bf16, tag=f"agA{f}_{dt_i}",
                        name=f"agA{f}_{dt_i}",
                    )
            agB_t = [[None] * 2 for _ in range(2)]  # [g][dt] -> [128, 1024]
            for g in range(2):
                for dt_i in range(2):
                    agB_t[g][dt_i] = ag_p.tile(
                        [128, 1024], bf16, tag=f"agB{g}_{dt_i}",
                        name=f"agB{g}_{dt_i}",
                    )

            # ---- output projection chunk: 512 cols, borrows a PSUM slot ----
            def outproj_chunk(f):
                g, part = f // 2, f % 2
                pt = strip_alloc("po")
                for m0, mw, pc, bc in ((0, 128, 0, 0), (128, 64, 512, 1)):
                    for dt_i in range(4):
                        nc.tensor.matmul(
                            pt[0:mw, pc : pc + 512],
                            wout_t[dt_i][:, m0 : m0 + mw],
                            agA_t[f][dt_i][:, :],
                            start=(dt_i == 0), stop=False,
                            skip_group_check=True,
                        )
                    for di, dt_i in enumerate((4, 5)):
                        nc.tensor.matmul(
                            pt[0:mw, pc : pc + 512],
                            wout_t[dt_i][:, m0 : m0 + mw],
                            agB_t[g][dt_i - 4][:, 512 * part : 512 * part + 512],
                            start=False, stop=(di == 1),
                            skip_group_check=True,
                        )
                    ot = o_p.tile([128, 512], f32, tag=f"oc{bc}")
                    nc.vector.tensor_scalar_add(
                        ot[0:mw, :], pt[0:mw, pc : pc + 512], bout_c[0:mw, bc : bc + 1]
                    )
                    nc.sync.dma_start(
                        out[m0 : m0 + mw, 512 * f : 512 * (f + 1)], ot[0:mw, :]
                    )

            # ---- wave B: head 2, ki pairs in row groups; AV pipelined ----
            # av2 folded: q cols [0,1024) in partitions 0:64,
            #             q cols [1024,2048) in partitions 64:128.
            def waveB(av_ps, atn):
                def flushB(f):
                    phalf = 64 * (f // 2)
                    c0 = 512 * (f % 2)
                    nc.vector.tensor_copy(
                        atn[:, 512 * f : 512 * (f + 1)],
                        av_ps[phalf : phalf + 64, c0 : c0 + 512],
                    )
                    if f % 2 == 0:
                        return
                    g = f // 2
                    gcols = slice(1024 * g, 1024 * (g + 1))
                    nc.sync.dma_start(ag_inB[g][:, :], atn[:, gcols])
                    nc.gpsimd.collective_compute(
                        "AllGather",
                        mybir.AluOpType.bypass,
                        replica_groups=groups,
                        ins=[ag_inB[g].ap().opt()],
                        outs=[ag_outB[g].ap().opt()],
                    )
                    for dt_i in range(2):
                        nc.sync.dma_start(
                            agB_t[g][dt_i][:, :],
                            ag_outB[g][dt_i * 128 : (dt_i + 1) * 128, :],
                        )

                def scores_batch(t, filler=None):
                    kis = (2 * t, 2 * t + 1)
                    srcs = {
                        kis[0]: (k2, 0, qk2, 0),
                        kis[1]: (qk2, 64, q2d, 64),
                    }
                    ets = {ki: [] for ki in kis}
                    accs = {ki: [] for ki in kis}
                    maxhv = max(len(halves_of(ki)) for ki in kis)
                    for hv in range(maxhv):
                        # PE filler between strip batches so the exps of the
                        # previous batch overlap matmul work
                        if hv == 1 and filler is not None:
                            filler()
                            filler = None
                        batch = []
                        for ki in kis:
                            hs = halves_of(ki)
                            if hv < len(hs):
                                batch.append((ki, hs[hv]))
                        s_list = []
                        for ki, (h0, hl) in batch:
                            s_ps = strip_alloc("spa")
                            s_list.append(s_ps)
                        maxhl = max(hl for _, (_, hl) in batch)
                        off = 0
                        while off < maxhl:
                            for s_ps, (ki, (h0, hl)) in zip(s_list, batch):
                                if off >= hl:
                                    continue
                                n = min(512, hl - off)
                                kT, kb, qT, qb = srcs[ki]
                                nc.tensor.matmul(
                                    s_ps[:, off : off + n],
                                    kT[kb : kb + 64, ki * 128 : (ki + 1) * 128],
                                    qT[qb : qb + 64, h0 + off : h0 + off + n],
                                    start=True,
                                    stop=True,
                                    skip_group_check=True,
                                )
                            off += 512
                        for s_ps, (ki, (h0, hl)) in zip(s_list, batch):
                            if causal and hv == 0:
                                nc.vector.tensor_add(
                                    s_ps[:, 0:128], s_ps[:, 0:128], tri_f[:]
                                )
                        for s_ps, (ki, (h0, hl)) in zip(s_list, batch):
                            et = e_p.tile([128, NMAX], bf16, tag="e")
                            acc = st_p.tile([128, 1], f32, tag="acc")
                            nc.scalar.activation(
                                et[:, 0:hl], s_ps[:, 0:hl], EXP,
                                scale=SCALE, accum_out=acc[:],
                            )
                            ets[ki].append((et, h0, hl))
                            accs[ki].append(acc)
                    if filler is not None:
                        filler()
                    return kis, ets, accs

                def do_av(kis, ets, accs):
                    for ki in kis:
                        vpt = make_vpt(accs[ki], ki, 2)
                        for et, h0, hl in ets[ki]:
                            # split chunks at the absolute-1024 boundary
                            a = h0
                            while a < h0 + hl:
                                lim = 1024 if a < 1024 else 2048
                                n = min(512, h0 + hl - a, lim - a)
                                phalf = 0 if a < 1024 else 64
                                c0 = a - (1024 if phalf else 0)
                                nc.tensor.matmul(
                                    av_ps[phalf : phalf + 64, c0 : c0 + n],
                                    vpt[:],
                                    et[:, a - h0 : a - h0 + n],
                                    start=(ki == 0),
                                    stop=(ki == 15),
                                    skip_group_check=True,
                                )
                                a += n

                pq = []  # 2-deep AV pipeline: chain latency ~2 t-steps
                for t in range(8):
                    fil = (lambda p=pq[0]: do_av(*p)) if len(pq) == 2 else None
                    cur = scores_batch(t, fil)
                    if fil is not None:
                        pq.pop(0)
                        pt_ = t - 2
                        if causal and pt_ % 2 == 1:
                            flushB(pt_ // 2)
                    if t % 2 == 0 and t < 6:
                        v_slot(t // 2 + 1, vnat)
                    pq.append(cur)
                for i, p in enumerate(pq):
                    do_av(*p)
                    pt_ = 6 + i
                    if causal and pt_ % 2 == 1:
                        flushB(pt_ // 2)
                if not causal:
                    for f in range(4):
                        flushB(f)

            # ---- wave A: heads 0+1 row/col paired, AV pipelined ----
            def waveA(av_lo, av_hi, atn, free_lo):
                def avcols(c0, cn):
                    if c0 < 1024:
                        return av_lo[:, c0 : c0 + cn]
                    return av_hi[:, c0 - 1024 : c0 - 1024 + cn]

                def flushA(f):
                    cols = slice(512 * f, 512 * (f + 1))
                    nc.vector.tensor_copy(atn[:, cols], avcols(512 * f, 512))
                    nc.sync.dma_start(ag_inA[f][:, :], atn[:, cols])
                    nc.gpsimd.collective_compute(
                        "AllGather",
                        mybir.AluOpType.bypass,
                        replica_groups=groups,
                        ins=[ag_inA[f].ap().opt()],
                        outs=[ag_outA[f].ap().opt()],
                    )
                    for dt_i in range(4):
                        nc.sync.dma_start(
                            agA_t[f][dt_i][:, :],
                            ag_outA[f][dt_i * 128 : (dt_i + 1) * 128, :],
                        )

                def do_av(ki, hs, ets, accs):
                    vpts = [make_vpt(accs[hi], ki, hi) for hi in range(2)]
                    for hv, (h0, hl) in enumerate(hs):
                        a = h0
                        while a < h0 + hl:
                            lim = 1024 if a < 1024 else 2048
                            n = min(512, h0 + hl - a, lim - a)
                            tgt = av_lo if a < 1024 else av_hi
                            c0 = a if a < 1024 else a - 1024
                            for hi in range(2):
                                p_lo = 0 if hi == 0 else 64
                                et = ets[hi][hv][0]
                                nc.tensor.matmul(
                                    tgt[p_lo : p_lo + 64, c0 : c0 + n],
                                    vpts[hi][:],
                                    et[:, a - h0 : a - h0 + n],
                                    start=(ki == 0),
                                    stop=(ki == 15),
                                    skip_group_check=True,
                                )
                            a += n

                def flushA3(half):
                    cols = slice(1536 + 256 * half, 1792 + 256 * half)
                    nc.vector.tensor_copy(
                        atn[:, cols], avcols(1536 + 256 * half, 256)
                    )
                    nc.sync.dma_start(ag_inA3h[half][:, :], atn[:, cols])
                    nc.gpsimd.collective_compute(
                        "AllGather",
                        mybir.AluOpType.bypass,
                        replica_groups=groups,
                        ins=[ag_inA3h[half].ap().opt()],
                        outs=[ag_outA3h[half].ap().opt()],
                    )
                    for dt_i in range(4):
                        nc.sync.dma_start(
                            agA_t[3][dt_i][:, 256 * half : 256 * half + 256],
                            ag_outA3h[half][dt_i * 128 : (dt_i + 1) * 128, :],
                        )

                oproj_at = {5: 0, 9: 1, 13: 2} if causal else {}
                pq = []  # 2-deep AV pipeline (pki = ki - 2)
                for ki in range(16):
                    hs = halves_of(ki)
                    ets = {0: [], 1: []}
                    accs = {0: [], 1: []}
                    done_av = len(pq) < 2
                    for hv, (h0, hl) in enumerate(hs):
                        # AV of ki-2 between this ki's strip batches:
                        # PE filler while the exps run
                        if hv == 1 and not done_av:
                            do_av(ki - 2, *pq.pop(0))
                            done_av = True
                        res = strip_pair(
                            ki,
                            [(k01, 0, q01, 0), (k01, 64, q01, 64)],
                            hv, h0, hl,
                        )
                        for hi, (et, acc) in enumerate(res):
                            ets[hi].append((et, h0, hl))
                            accs[hi].append(acc)
                    if not done_av:
                        do_av(ki - 2, *pq.pop(0))
                        done_av = True
                    if len(pq) == 1 and ki >= 2:
                        pki = ki - 2
                        if causal and pki % 4 == 3:
                            flushA(pki // 4)
                            if pki == 7:
                                free_lo()
                        if causal and pki == 13:
                            flushA3(0)
                        if pki in oproj_at:
                            outproj_chunk(oproj_at[pki])
                    pq.append((hs, ets, accs))
                for i, p in enumerate(pq):
                    pki = 14 + i
                    do_av(pki, *p)
                if causal:
                    flushA3(1)
                    outproj_chunk(3)
                else:
                    for f in range(4):
                        flushA(f)
                    for f in range(4):
                        outproj_chunk(f)

            # ---- run: waveB (with v slots 1..3 + v slot 0 upfront) ----
            psB_ctx = tc.tile_pool(name="psB", bufs=1, space="PSUM")
            psB = psB_ctx.__enter__()
            av2 = psB.tile([128, 1024], f32, tag="av2")
            atn2 = atn_p.tile([64, S], bf16, tag="atn2")
            v_slot(0, vnat)
            waveB(av2, atn2)
            psB_ctx.__exit__(None, None, None)
            xs_ctx.__exit__(None, None, None)
            xt_ctx.__exit__(None, None, None)

            # waveA: 2-buf strip pool + lo/hi accumulator halves; the lo
            # half's 2 banks become a 3rd strip buffer after its flush
            psS_ctx.__exit__(None, None, None)
            psS2_ctx = tc.tile_pool(name="psS2", bufs=2, space="PSUM")
            psS = psS2_ctx.__enter__()
            psAh_ctx = tc.tile_pool(name="psAh", bufs=1, space="PSUM")
            psAh = psAh_ctx.__enter__()
            avA_hi = psAh.tile([128, 1024], f32, tag="avAh")
            psAl_ctx = tc.tile_pool(name="psAl", bufs=1, space="PSUM")
            psAl = psAl_ctx.__enter__()
            avA_lo = psAl.tile([128, 1024], f32, tag="avAl")
            psSx_ctx = tc.tile_pool(name="psSx", bufs=1, space="PSUM")
            hand = {"done": False}

            def free_lo():
                psAl_ctx.__exit__(None, None, None)
                strip_state["extra"] = psSx_ctx.__enter__()
                hand["done"] = True

            atnA = atn_p.tile([128, S], bf16, tag="atnA")
            waveA(avA_lo, avA_hi, atnA, free_lo)
            if hand["done"]:
                strip_state["extra"] = None
                psSx_ctx.__exit__(None, None, None)
            else:
                psAl_ctx.__exit__(None, None, None)
            psAh_ctx.__exit__(None, None, None)
            psS2_ctx.__exit__(None, None, None)
    nc.compile()
    return nc


def _shards(x, mask, W_in, b_in, W_out, b_out):
    """Build per-core input maps (host-side sharding / layout prep)."""
    tri_np = np.where(
        np.arange(128)[None, :] < np.arange(128)[:, None], np.float32(NEG), 0.0
    ).astype(np.float32)
    # split-AllGather row order: rank pairs (h=3r,3r+1) then solos (h=3r+2)
    head_order = [0, 1, 3, 4, 6, 7, 9, 10, 2, 5, 8, 11]
    row_perm = np.concatenate([np.arange(h * 64, (h + 1) * 64) for h in head_order])
    in_maps = []
    for c in range(NCORES):
        b = c // GROUPS
        g = c % GROUPS
        hs = [3 * g, 3 * g + 1, 3 * g + 2]
        qc = [W_in[:, 64 * h : 64 * (h + 1)] for h in hs]
        kc = [W_in[:, D + 64 * h : D + 64 * (h + 1)] for h in hs]
        vc = W_in[:, 2 * D + 64 * hs[0] : 2 * D + 64 * (hs[2] + 1)]
        qb = [b_in[64 * h : 64 * (h + 1)] for h in hs]
        kb = [b_in[D + 64 * h : D + 64 * (h + 1)] for h in hs]
        vb = b_in[2 * D + 64 * hs[0] : 2 * D + 64 * (hs[2] + 1)]
        wqk = np.concatenate(
            [kc[0], kc[1], qc[0], qc[1], qc[2], kc[2]], axis=1
        ).astype(np.float32)
        bqk = np.concatenate([kb[0], kb[1], qb[0], qb[1], qb[2], kb[2]])
        in_maps.append(
            {
                "xT": np.ascontiguousarray(x[b].T, dtype=np.float32),
                "wqk": np.ascontiguousarray(wqk),
                "wv": np.ascontiguousarray(vc, dtype=np.float32),
                "bqkc": np.ascontiguousarray(bqk[:, None], dtype=np.float32),
                "bv4": np.ascontiguousarray(
                    np.tile(vb, 4)[None, :], dtype=np.float32
                ),
                "wout": np.ascontiguousarray(
                    W_out[row_perm, EPG * g : EPG * (g + 1)], dtype=np.float32
                ),
                "boutc": np.ascontiguousarray(
                    b_out[EPG * g : EPG * (g + 1), None], dtype=np.float32
                ),
                "tri": tri_np,
                "triT": np.ascontiguousarray(tri_np.T),
                "ident": np.eye(128, dtype=np.float32),
            }
        )
    return in_maps


def _numpy_ref(x, mask, W_in, b_in, W_out, b_out):
    qkv = x @ W_in + b_in
    q, k, v = np.split(qkv, 3, axis=2)
    q = q.reshape(B, S, H, DH).transpose(0, 2, 1, 3)
    k = k.reshape(B, S, H, DH).transpose(0, 2, 1, 3)
    v = v.reshape(B, S, H, DH).transpose(0, 2, 1, 3)
    attn = np.einsum("bhqd,bhkd->bhqk", q, k) / np.sqrt(np.float32(D))
    attn = np.where(mask == 0, -np.inf, attn)
    attn = attn - attn.max(axis=-2, keepdims=True)
    e = np.exp(attn)
    attn = e / e.sum(axis=-2, keepdims=True)
    out = np.einsum("bhqk,bhkd->bhqd", attn, v)
    out = out.transpose(0, 2, 1, 3).reshape(B, S, D)
    return (out @ W_out + b_out).astype(np.float32)


def _run(inputs, trace=False):
    from concourse.bass_utils import run_bass_kernel_spmd

    x = np.asarray(inputs["x"], dtype=np.float32)
    mask = np.asarray(inputs["mask"])
    W_in = np.asarray(inputs["W_in"], dtype=np.float32)
    b_in = np.asarray(inputs["b_in"], dtype=np.float32)
    W_out = np.asarray(inputs["W_out"], dtype=np.float32)
    b_out = np.asarray(inputs["b_out"], dtype=np.float32)

    m2 = np.asarray(mask).reshape(S, S)
    if np.array_equal(m2, np.tril(np.ones((S, S), m2.dtype))):
        causal = True
    elif np.array_equal(m2, np.ones((S, S), m2.dtype)):
        causal = False
    else:
        return _numpy_ref(x, mask, W_in, b_in, W_out, b_out), None

    key = ("nc", causal)
    if key not in _cache:
        _cache[key] = _build(causal)
    nc = _cache[key]

    in_maps = _shards(x, mask, W_in, b_in, W_out, b_out)
    res = run_bass_kernel_spmd(nc, in_maps, core_ids=list(range(NCORES)), trace=trace)

    full = np.empty((B, S, D), dtype=np.float32)
    for c in range(NCORES):
        b, g = c // GROUPS, c % GROUPS
        full[b, :, EPG * g : EPG * (g + 1)] = res.results[c]["out"].T
    return full, res


def kernel(**inputs) -> np.ndarray:
    out, _ = _run(inputs, trace=False)
    return out
